# revision 5
# baseline (speedup 1.0000x reference)
"""Trainium2 Bass kernel for Cos_RootHist_GLM test_forward.

Pipeline (two NEFF launches):
  1. phase NEFF, SPMD over 8 cores, time-sharded with 256-step halo:
     spike->subunit projection (PE matmuls), per-subunit 200-tap causal
     FIR via Toeplitz matmuls (PE), subunit-tree softplus (Exp+Ln on ACT),
     per-core ns_out / b / out_filters.
  2. scan NEFF, single core: the 20000-step nonlinear autoregressive
     root recurrence.  Serial chain on ACT (Identity-merge, Exp, Ln per
     step; softplus(x) = ln(1+exp(x)) since no softplus table exists),
     with the 200-tap history split: lag 2 inside the chain op, lags
     3..34 via per-step DVE scalar_tensor_tensor rank-1 updates into an
     SBUF ring accumulator, lags 35..200 via per-32-step PE Toeplitz
     matmuls accumulating into a PSUM ring pre-initialized with b.
"""

import functools
import math
import numpy as np

# ---------------------------------------------------------------------------
# activation-table patch: the act-table-load pass picks the first table
# containing each function, which alternates exp_and_others/natural_log for
# an Exp-Ln chain and inserts a ~1.3us table load before every activation.
# Restricting candidates to natural_log_exp_and_others (order preserved, so
# act_func_set_ids stay valid) yields a single load.
import concourse.hw_specs as _hw_specs
import concourse.bacc as _bacc_mod

_orig_get_tables = _hw_specs.get_activation_tables.__wrapped__


@functools.cache
def _patched_tables(module_arch):
    tables = dict(_orig_get_tables(module_arch))
    keep = "natural_log_exp_and_others"
    assert keep in tables
    return {k: (v if k == keep else set()) for k, v in tables.items()}


_hw_specs.get_activation_tables = _patched_tables
_bacc_mod.get_activation_tables = _patched_tables

import concourse.bass as bass
import concourse.bacc as bacc
import concourse.mybir as mybir
import concourse.tile as tile
from concourse.bass_utils import run_bass_kernel_spmd

F32 = mybir.dt.float32
EXP = mybir.ActivationFunctionType.Exp
LN = mybir.ActivationFunctionType.Ln
IDN = mybir.ActivationFunctionType.Identity
MUL = mybir.AluOpType.mult
ADD = mybir.AluOpType.add

SUB_NO = 15
T_NO = 200
N_E = 2000
N_I = 500
T_DATA = 20000
NB = 19
NCORES = 8
HALO = 256          # 2 column-tiles of 128; >= T_NO - 1 = 199
SHARD = T_DATA // NCORES          # 2500
TW = SHARD + HALO                 # 2756 rows of spikes per core
COLS = (TW + 127) // 128          # 22 column tiles
TWP = COLS * 128                  # 2816 padded rows

# scan constants
NEAR_W = 64          # lags 3..66 on DVE
FAR_LO = NEAR_W + 3  # 67: first PE lag
FAR_W = T_NO - FAR_LO + 1 + 31    # Toeplitz width per 32-block: 165
ACCA_RING = 2048
SU_RING = 512
YPAD = 8


def _build_phase_neff():
    nc = bacc.Bacc("TRN2", target_bir_lowering=False, debug=False)
    SeT = nc.dram_tensor("SeT", [2048, TWP], F32, kind="ExternalInput").ap()
    SiT = nc.dram_tensor("SiT", [512, TWP], F32, kind="ExternalInput").ap()
    Ce = nc.dram_tensor("Ce", [2048, SUB_NO], F32, kind="ExternalInput").ap()
    Ci = nc.dram_tensor("Ci", [512, SUB_NO], F32, kind="ExternalInput").ap()
    TOEP = nc.dram_tensor("TOEP", [90, 128, 128], F32, kind="ExternalInput").ap()
    ThetaR = nc.dram_tensor("ThetaR", [128, SUB_NO], F32, kind="ExternalInput").ap()
    W2R = nc.dram_tensor("W2R", [128, SUB_NO], F32, kind="ExternalInput").ap()
    WeT = nc.dram_tensor("WeT", [NB, SUB_NO], F32, kind="ExternalInput").ap()
    WiT = nc.dram_tensor("WiT", [NB, SUB_NO], F32, kind="ExternalInput").ap()
    WhT = nc.dram_tensor("WhT", [NB, 1], F32, kind="ExternalInput").ap()
    COSB = nc.dram_tensor("COSB", [NB, T_NO], F32, kind="ExternalInput").ap()

    NSOUT = nc.dram_tensor("NSOUT", [SHARD, SUB_NO], F32, kind="ExternalOutput").ap()
    BOUT = nc.dram_tensor("BOUT", [SHARD], F32, kind="ExternalOutput").ap()
    FILT = nc.dram_tensor("FILT", [2 * SUB_NO + 1, T_NO], F32, kind="ExternalOutput").ap()

    with tile.TileContext(nc) as tc:
        with tc.tile_pool(name="const", bufs=1) as cpool, \
             tc.tile_pool(name="work", bufs=3) as wpool, \
             tc.tile_pool(name="psum", bufs=2, space="PSUM") as ppool, \
             tc.tile_pool(name="psumf", bufs=1, space="PSUM") as fpool:
            ces = cpool.tile([128, 16 * SUB_NO], F32)
            cis = cpool.tile([128, 4 * SUB_NO], F32)
            for k in range(16):
                nc.sync.dma_start(out=ces[:, 15 * k:15 * (k + 1)], in_=Ce[128 * k:128 * (k + 1), :])
            for k in range(4):
                nc.sync.dma_start(out=cis[:, 15 * k:15 * (k + 1)], in_=Ci[128 * k:128 * (k + 1), :])
            toep = cpool.tile([128, 90 * 128], F32)
            # dst (q, m, p'): src (m, q, p')
            toep_v = toep[:, :].rearrange("q (m p) -> q m p", m=90)
            nc.sync.dma_start(out=toep_v, in_=bass.AP(
                tensor=TOEP.tensor, offset=TOEP.offset,
                ap=[[128, 128], [128 * 128, 90], [1, 128]]))
            thetar = cpool.tile([128, SUB_NO], F32)
            nc.sync.dma_start(out=thetar, in_=ThetaR)
            w2r = cpool.tile([128, SUB_NO], F32)
            nc.sync.dma_start(out=w2r, in_=W2R)

            # filters output (tiny)
            wet = cpool.tile([NB, SUB_NO], F32)
            nc.sync.dma_start(out=wet, in_=WeT)
            wit = cpool.tile([NB, SUB_NO], F32)
            nc.sync.dma_start(out=wit, in_=WiT)
            wht = cpool.tile([NB, 1], F32)
            nc.sync.dma_start(out=wht, in_=WhT)
            cosb = cpool.tile([NB, T_NO], F32)
            nc.sync.dma_start(out=cosb, in_=COSB)
            filt = cpool.tile([SUB_NO, 3 * T_NO], F32)
            pf = fpool.tile([SUB_NO, T_NO], F32, tag="pf")
            nc.tensor.matmul(pf, wet, cosb, start=True, stop=True)
            nc.vector.tensor_copy(filt[:, 0:T_NO], pf)
            pf2 = fpool.tile([SUB_NO, T_NO], F32, tag="pf")
            nc.tensor.matmul(pf2, wit, cosb, start=True, stop=True)
            nc.vector.tensor_copy(filt[:, T_NO:2 * T_NO], pf2)
            pf3 = fpool.tile([1, T_NO], F32, tag="pf")
            nc.tensor.matmul(pf3, wht, cosb, start=True, stop=True)
            nc.vector.tensor_copy(filt[0:1, 2 * T_NO:3 * T_NO], pf3)
            nc.sync.dma_start(out=FILT[0:SUB_NO, :], in_=filt[:, 0:T_NO])
            nc.sync.dma_start(out=FILT[SUB_NO:2 * SUB_NO, :], in_=filt[:, T_NO:2 * T_NO])
            nc.sync.dma_start(out=FILT[2 * SUB_NO:2 * SUB_NO + 1, :], in_=filt[0:1, 2 * T_NO:3 * T_NO])

            # A: raw_g[t, s] = sum_n S[t, n] C[s, n]  -> [128, 15] per t-tile
            rawe = cpool.tile([128, SUB_NO * COLS], F32)
            rawi = cpool.tile([128, SUB_NO * COLS], F32)
            for tt in range(COLS):
                pe = ppool.tile([128, SUB_NO], F32, tag="pa")
                for k in range(16):
                    st = wpool.tile([128, 128], F32, tag="sload")
                    nc.sync.dma_start(out=st, in_=SeT[128 * k:128 * (k + 1), 128 * tt:128 * (tt + 1)])
                    nc.tensor.matmul(pe, st, ces[:, 15 * k:15 * (k + 1)],
                                     start=(k == 0), stop=(k == 15))
                nc.vector.tensor_copy(
                    bass.AP(tensor=rawe.tensor, offset=rawe.offset + tt,
                            ap=[[SUB_NO * COLS, 128], [COLS, SUB_NO]]), pe)
                pi = ppool.tile([128, SUB_NO], F32, tag="pa")
                for k in range(4):
                    st = wpool.tile([128, 128], F32, tag="sload")
                    nc.sync.dma_start(out=st, in_=SiT[128 * k:128 * (k + 1), 128 * tt:128 * (tt + 1)])
                    nc.tensor.matmul(pi, st, cis[:, 15 * k:15 * (k + 1)],
                                     start=(k == 0), stop=(k == 3))
                nc.vector.tensor_copy(
                    bass.AP(tensor=rawi.tensor, offset=rawi.offset + tt,
                            ap=[[SUB_NO * COLS, 128], [COLS, SUB_NO]]), pi)

            # C: conv per subunit: syn_s[:, c] = sum_j T_j^(s) @ raw_s[:, c-j]
            syn = cpool.tile([128, SUB_NO * COLS], F32)
            for s in range(SUB_NO):
                pc = ppool.tile([128, COLS], F32, tag="pconv")
                first = True
                for g, raw in ((0, rawe), (1, rawi)):
                    for j in range(3):
                        m = g * 45 + s * 3 + j
                        lhsT = toep[:, 128 * m:128 * (m + 1)]
                        rhs = bass.AP(tensor=raw.tensor, offset=raw.offset + s * COLS,
                                      ap=[[SUB_NO * COLS, 128], [1, COLS - j]])
                        nc.tensor.matmul(pc[:, j:COLS], lhsT, rhs,
                                         start=first, stop=(g == 1 and j == 2))
                        first = False
                nc.vector.tensor_copy(syn[:, s * COLS:(s + 1) * COLS], pc)

            # D: tree (children of p are 2p+1, 2p+2), j = 14..1
            ns = cpool.tile([128, SUB_NO * COLS], F32)
            nc.vector.memset(ns[:, 0:COLS], 0.0)
            tmp = cpool.tile([128, COLS], F32)
            uu = cpool.tile([128, COLS], F32)
            for j in range(SUB_NO - 1, 0, -1):
                c1, c2 = 2 * j + 1, 2 * j + 2
                src = syn[:, j * COLS:(j + 1) * COLS]
                if c1 < SUB_NO:
                    nc.vector.scalar_tensor_tensor(out=tmp, in0=ns[:, c1 * COLS:(c1 + 1) * COLS],
                                                   scalar=w2r[:, c1:c1 + 1], in1=src,
                                                   op0=MUL, op1=ADD)
                    src = tmp
                if c2 < SUB_NO:
                    nc.vector.scalar_tensor_tensor(out=tmp, in0=ns[:, c2 * COLS:(c2 + 1) * COLS],
                                                   scalar=w2r[:, c2:c2 + 1], in1=src,
                                                   op0=MUL, op1=ADD)
                    src = tmp
                nc.scalar.activation(out=uu, in_=src, func=EXP, scale=1.0,
                                     bias=thetar[:, j:j + 1])
                nc.scalar.activation(out=ns[:, j * COLS:(j + 1) * COLS], in_=uu,
                                     func=LN, scale=1.0, bias=1.0)

            # b = syn[:,0] + w2[1]*ns1 + w2[2]*ns2 + Theta[0]
            bt = cpool.tile([128, COLS], F32)
            nc.vector.scalar_tensor_tensor(out=bt, in0=ns[:, COLS:2 * COLS],
                                           scalar=w2r[:, 1:2], in1=syn[:, 0:COLS],
                                           op0=MUL, op1=ADD)
            nc.vector.scalar_tensor_tensor(out=bt, in0=ns[:, 2 * COLS:3 * COLS],
                                           scalar=w2r[:, 2:3], in1=bt,
                                           op0=MUL, op1=ADD)
            nc.scalar.activation(out=bt, in_=bt, func=IDN, scale=1.0,
                                 bias=thetar[:, 0:1])

            # exports: local t = 128*col + p ; output rows HALO..HALO+2499
            # full cols 2..20 (19 cols), partial col 21 rows 0..67
            nsx = cpool.tile([128, COLS * SUB_NO], F32)  # (col, s) layout
            for s in range(SUB_NO):
                nc.vector.tensor_copy(
                    bass.AP(tensor=nsx.tensor, offset=nsx.offset + s,
                            ap=[[COLS * SUB_NO, 128], [SUB_NO, COLS]]),
                    ns[:, s * COLS:(s + 1) * COLS])
            nfull = (SHARD + HALO) // 128 - 2      # 19 full cols after halo
            rem = SHARD - nfull * 128              # 68
            nc.sync.dma_start(
                out=bass.AP(tensor=NSOUT.tensor, offset=NSOUT.offset,
                            ap=[[SUB_NO, 128], [128 * SUB_NO, nfull], [1, SUB_NO]]),
                in_=bass.AP(tensor=nsx.tensor, offset=nsx.offset + 2 * SUB_NO,
                            ap=[[COLS * SUB_NO, 128], [SUB_NO, nfull], [1, SUB_NO]]))
            nc.sync.dma_start(
                out=bass.AP(tensor=NSOUT.tensor, offset=NSOUT.offset + nfull * 128 * SUB_NO,
                            ap=[[SUB_NO, rem], [1, SUB_NO]]),
                in_=bass.AP(tensor=nsx.tensor, offset=nsx.offset + (2 + nfull) * SUB_NO,
                            ap=[[COLS * SUB_NO, rem], [1, SUB_NO]]))
            nc.sync.dma_start(
                out=bass.AP(tensor=BOUT.tensor, offset=BOUT.offset,
                            ap=[[1, 128], [128, nfull]]),
                in_=bass.AP(tensor=bt.tensor, offset=bt.offset + 2,
                            ap=[[COLS, 128], [1, nfull]]))
            nc.sync.dma_start(
                out=bass.AP(tensor=BOUT.tensor, offset=BOUT.offset + nfull * 128,
                            ap=[[1, rem]]),
                in_=bass.AP(tensor=bt.tensor, offset=bt.offset + 2 + nfull,
                            ap=[[COLS, rem]]))
    nc.compile()
    return nc


def _build_scan_neff(T=T_DATA):
    nc = bacc.Bacc("TRN2", target_bir_lowering=False, debug=False)
    B = nc.dram_tensor("B", [T], F32, kind="ExternalInput").ap()
    HN = nc.dram_tensor("HN", [1, NEAR_W], F32, kind="ExternalInput").ap()
    H32 = nc.dram_tensor("H32", [32, FAR_W], F32, kind="ExternalInput").ap()
    SC = nc.dram_tensor("SC", [1, 4], F32, kind="ExternalInput").ap()  # c2, w2_0, V_o, 1.0
    FV = nc.dram_tensor("FV", [T], F32, kind="ExternalOutput").ap()

    with tile.TileContext(nc) as tc:
        with tc.tile_pool(name="sb", bufs=1) as sb, \
             tc.tile_pool(name="ps", bufs=4, space="PSUM") as ps:
            Y = sb.tile([1, YPAD + T], F32)
            bS = sb.tile([1, T], F32)
            ACCA = sb.tile([1, ACCA_RING], F32)
            Ur = sb.tile([1, SU_RING], F32)
            hn = sb.tile([1, NEAR_W], F32)
            h32 = sb.tile([32, FAR_W], F32)
            sc = sb.tile([1, 4], F32)
            ones = sb.tile([1, 1], F32)

            nc.sync.dma_start(out=bS[0:1, 0:T],
                              in_=bass.AP(tensor=B.tensor, offset=B.offset, ap=[[T, 1], [1, T]]))
            nc.sync.dma_start(out=hn, in_=HN)
            nc.sync.dma_start(out=h32, in_=H32)
            nc.sync.dma_start(out=sc, in_=SC)
            nc.vector.memset(ones, 1.0)
            nc.vector.memset(Y[0:1, 0:YPAD], 0.0)
            # ACCA prologue: slots [0, 2048) = b (clipped; rest zero)
            w0 = min(ACCA_RING, T)
            nc.vector.tensor_copy(ACCA[0:1, 0:w0], bS[0:1, 0:w0])
            if w0 < ACCA_RING:
                nc.vector.memset(ACCA[0:1, w0:ACCA_RING], 0.0)

            c2ap = sc[0:1, 0:1]

            def acca_rmw(t0, width, in0_ap, scalar_ap):
                # ACCA[t0:t0+width (ring)] += in0 * scalar, split on ring wrap
                done = 0
                while done < width:
                    a0 = (t0 + done) % ACCA_RING
                    w = min(width - done, ACCA_RING - a0)
                    nc.vector.scalar_tensor_tensor(
                        out=ACCA[0:1, a0:a0 + w],
                        in0=in0_ap[0:1, done:done + w],
                        scalar=scalar_ap,
                        in1=ACCA[0:1, a0:a0 + w], op0=MUL, op1=ADD)
                    done += w

            for i in range(T):
                si = i % SU_RING
                ai = i % ACCA_RING
                if i % 256 == 0 and i >= 256:
                    # recycle ACCA slots for targets [i+512, i+768) := b
                    t0 = i + 512
                    if t0 < T:
                        z0 = t0 % ACCA_RING
                        hi = min(t0 + 256, T)
                        nc.vector.tensor_copy(ACCA[0:1, z0:z0 + (hi - t0)], bS[0:1, t0:hi])
                # E(i) = Exp(Y[i-2]*c2 + ACCA[i]);  L(i): Y[i] = Ln(U + 1)
                nc.scalar.activation(out=Ur[0:1, si:si + 1],
                                     in_=Y[0:1, YPAD + i - 2:YPAD + i - 1],
                                     func=EXP, scale=c2ap, bias=ACCA[0:1, ai:ai + 1])
                nc.scalar.activation(out=Y[0:1, YPAD + i:YPAD + i + 1],
                                     in_=Ur[0:1, si:si + 1],
                                     func=LN, scale=1.0, bias=1.0)
                # near feeder from source i: targets [i+3, i+3+NEAR_W) clipped
                w = min(NEAR_W, T - (i + 3))
                if w > 0:
                    yi = Y[0:1, YPAD + i:YPAD + i + 1]
                    done = 0
                    while done < w:
                        a0 = (i + 3 + done) % ACCA_RING
                        w1 = min(w - done, ACCA_RING - a0)
                        nc.vector.scalar_tensor_tensor(
                            out=ACCA[0:1, a0:a0 + w1], in0=hn[0:1, done:done + w1],
                            scalar=yi, in1=ACCA[0:1, a0:a0 + w1], op0=MUL, op1=ADD)
                        done += w1
                # PE far block every 32 steps: own PSUM strip, then DVE merge
                if i % 32 == 31 and i + 4 < T:
                    k0 = i - 31
                    ytp = ps.tile([32, 1], F32, tag="ytp")
                    nc.tensor.matmul(ytp, Y[0:1, YPAD + k0:YPAD + k0 + 32], ones,
                                     start=True, stop=True)
                    yts = sb.tile([32, 1], F32, tag="yts")
                    nc.vector.tensor_copy(yts, ytp)
                    fw = min(FAR_W, max(0, T - (k0 + FAR_LO)))
                    if fw > 0:
                        fblk = ps.tile([1, FAR_W], F32, tag="fblk")
                        nc.tensor.matmul(fblk[0:1, 0:fw], yts, h32[:, 0:fw],
                                         start=True, stop=True)
                        acca_rmw(k0 + FAR_LO, fw, fblk, 1.0)

            fvs = sb.tile([1, 2048], F32)
            for c0 in range(0, T, 2048):
                w = min(2048, T - c0)
                nc.vector.tensor_scalar(out=fvs[0:1, 0:w], in0=Y[0:1, YPAD + c0:YPAD + c0 + w],
                                        scalar1=sc[0:1, 1:2], scalar2=sc[0:1, 2:3],
                                        op0=MUL, op1=ADD)
                nc.sync.dma_start(out=bass.AP(tensor=FV.tensor, offset=FV.offset + c0, ap=[[T, 1], [1, w]]),
                                  in_=fvs[0:1, 0:w])
    nc.compile()
    return nc


def _host_prep(inputs):
    S_e = np.asarray(inputs["S_e"], np.float32)
    S_i = np.asarray(inputs["S_i"], np.float32)
    C_syn_e = np.asarray(inputs["C_syn_e"], np.float32)
    C_syn_i = np.asarray(inputs["C_syn_i"], np.float32)
    cos_basis = np.asarray(inputs["cos_basis"], np.float32)
    W_syn = np.asarray(inputs["W_syn"], np.float32)
    W_sub = np.asarray(inputs["W_sub"], np.float32)
    V_o = np.asarray(inputs["V_o"], np.float32)
    Theta = np.asarray(inputs["Theta"], np.float32)
    W_hist = np.asarray(inputs["W_hist"], np.float32)

    e_kern = (W_syn[:, :, 0] @ cos_basis).astype(np.float32)
    i_kern = (W_syn[:, :, 1] @ cos_basis).astype(np.float32)
    hist = (W_hist @ cos_basis).astype(np.float32)
    wsub2 = (W_sub ** 2).astype(np.float32)

    # Toeplitz (transposed for lhsT): TT[m][q, p] = kern[shift + p - q]
    def toep_T(kern_row, shift):
        q = np.arange(128)[:, None]
        p = np.arange(128)[None, :]
        d = shift + p - q
        valid = (d >= 0) & (d < T_NO)
        return np.where(valid, kern_row[np.clip(d, 0, T_NO - 1)], 0.0).astype(np.float32)

    TOEP = np.zeros((90, 128, 128), np.float32)
    for g, kern in ((0, e_kern), (1, i_kern)):
        for s in range(SUB_NO):
            for j in range(3):
                TOEP[g * 45 + s * 3 + j] = toep_T(kern[s], 128 * j)

    SeT_full = np.ascontiguousarray(S_e.T)  # [2000, 20000]
    SiT_full = np.ascontiguousarray(S_i.T)  # [500, 20000]

    per_core = []
    for c in range(NCORES):
        lo = c * SHARD - HALO
        hi = c * SHARD + SHARD
        se = np.zeros((2048, TWP), np.float32)
        si = np.zeros((512, TWP), np.float32)
        a, b_ = max(lo, 0), hi
        se[:N_E, (a - lo):(a - lo) + (b_ - a)] = SeT_full[:, a:b_]
        si[:N_I, (a - lo):(a - lo) + (b_ - a)] = SiT_full[:, a:b_]
        per_core.append({
            "SeT": se, "SiT": si,
            "Ce": np.vstack([C_syn_e.T, np.zeros((48, SUB_NO), np.float32)]),
            "Ci": np.vstack([C_syn_i.T, np.zeros((12, SUB_NO), np.float32)]),
            "TOEP": TOEP,
            "ThetaR": np.tile(Theta[None, :], (128, 1)).astype(np.float32),
            "W2R": np.tile(wsub2[None, :], (128, 1)).astype(np.float32),
            "WeT": np.ascontiguousarray(W_syn[:, :, 0].T),
            "WiT": np.ascontiguousarray(W_syn[:, :, 1].T),
            "WhT": np.ascontiguousarray(W_hist[:, None]),
            "COSB": cos_basis,
        })

    # scan consts
    HN = np.zeros((1, NEAR_W), np.float32)
    for w in range(NEAR_W):
        HN[0, w] = hist[2 + w]            # lag 3+w
    H32m = np.zeros((32, FAR_W), np.float32)
    for j in range(32):
        for n in range(FAR_W):
            lag = FAR_LO + n - j
            if FAR_LO <= lag <= T_NO:
                H32m[j, n] = hist[lag - 1]
    SC = np.array([[hist[1], wsub2[0], V_o[0], 1.0]], np.float32)
    return per_core, HN, H32m, SC, hist


_CACHE = {}
LAST_EXEC_NS = None


def kernel(**inputs):
    global LAST_EXEC_NS
    per_core, HN, H32m, SC, hist = _host_prep(inputs)

    if "phase" not in _CACHE:
        _CACHE["phase"] = _build_phase_neff()
    nc_p = _CACHE["phase"]
    res_p = run_bass_kernel_spmd(nc_p, per_core, core_ids=list(range(NCORES)))
    outs = res_p.results

    ns_out = np.concatenate([outs[c]["NSOUT"] for c in range(NCORES)], axis=0)
    b_full = np.concatenate([outs[c]["BOUT"] for c in range(NCORES)], axis=0).astype(np.float32)
    out_filters = np.asarray(outs[0]["FILT"], np.float32)

    if "scan" not in _CACHE:
        _CACHE["scan"] = _build_scan_neff(T_DATA)
    nc_s = _CACHE["scan"]
    res_s = run_bass_kernel_spmd(
        nc_s, [{"B": b_full, "HN": HN, "H32": H32m, "SC": SC}], core_ids=[0])
    final_V = np.asarray(res_s.results[0]["FV"], np.float32)

    t_p = res_p.exec_time_ns
    t_s = res_s.exec_time_ns
    LAST_EXEC_NS = (t_p or 0) + (t_s or 0) if (t_p or t_s) else None

    return final_V, np.asarray(ns_out, np.float32), out_filters
